# revision 66
# baseline (speedup 1.0000x reference)
"""Multi-head causal attention (B=2, T=2048, H=1024, NH=16) on 8 TRN2 cores.

Sharding: core c owns batch c//4 and heads 4*(c%4)..4*(c%4)+4 (tensor
parallel on heads, data parallel on batch). Each core projects Q/K/V for its
head slice (column parallel), runs causal attention for its 4 heads, applies
its w_o row slice to all tokens, and 4 token-chunked ReduceScatters sum the
partials across each 4-core head group.

Schedule: attention runs tq-chunk-major (4 chunks of 512 query tokens). As
soon as chunk n is attended + normalized + output-projected, its 512-token
ReduceScatter launches and overlaps the compute of chunk n+1; only the last
chunk's RS (15us launch + 0.25MB) plus the final output copy is exposed.
Within each chunk the two heads of a group run in lockstep (one 1024-wide exp
per tk step), PV lags scores by TWO steps (so the only cross-engine gate is
the 2-deep score-PSUM rotation), and Q/K/V/out-projection matmul "fillers"
ride a deadline-keyed, credit-metered queue that injects them between steps
wherever the scalar engine's exp would otherwise stall the tensor engine.

All matmul operands are fp16 (full PE rate at any tile size, so the
below-diagonal halves of boundary score blocks are skipped exactly); softmax
denominators come from an all-ones column appended to V, reciprocated
straight out of PSUM, broadcast by K=1 ones matmuls, and multiplied into the
attention output in place. b_v is folded into an effective output bias
host-side (b_o/4 + b_v_g @ w_oT_g, exact), which a single ones-matmul
broadcasts so the out-projection drain fuses the bias add. PSUM->SBUF drains
alternate between DVE and Act so neither queue paces the PE stream.
"""

import numpy as np

B, T, H, NH, HD = 2, 2048, 1024, 16, 64
NCORES = 8
GROUPS = 4  # head-groups == cores per batch
D = H // GROUPS  # 256 output dims per core
HPC = NH // GROUPS  # 4 heads per core
TS = T // GROUPS  # 512-token output slice per core
P = 128
KO = H // P  # 8 contraction chunks
NQ = T // 512  # 4 tq chunks of 512
NT = T // P  # 16 tk chunks of 128

_nc_cache = {}


def build_nc(reps: int = 1, body: str = "all"):
    """Build the per-core Bass program (identical across cores)."""
    import concourse.mybir as mybir
    import concourse.tile as tile
    from concourse import bacc

    f32 = mybir.dt.float32
    f32r = mybir.dt.float32r
    f16 = mybir.dt.float16
    AF = mybir.ActivationFunctionType
    ALU = mybir.AluOpType

    nc = bacc.Bacc("TRN2", target_bir_lowering=False, debug=False, num_devices=NCORES)

    def inp(name, shape, dt=f16):
        return nc.dram_tensor(name, shape, dt, kind="ExternalInput").ap()

    xq_ext = inp("xqT", [H, T])
    xk_ext = inp("xkT", [H, T])
    xv_ext = inp("xvT", [H, T])
    wq_ext = inp("wqT", [H, D])
    wk_ext = inp("wkT", [H, D])
    wv_ext = inp("wvT", [H, D])
    wo_ext = inp("woT", [D, H])
    bq_ext = inp("bq", [P, D // P], f32)
    bk_ext = inp("bk", [P, D // P], f32)
    boeff_ext = inp("boeff", [1, H], f32r)  # b_o/4 + b_v_g @ w_oT_g
    mask_ext = inp("mask128", [P, P])  # upper-tri (f >= p) diagonal-block mask
    ones_ext = inp("ones", [P, P])
    onesr_ext = inp("onesr", [1, P], f32r)
    out_ext = nc.dram_tensor("out", [TS, H], f16, kind="ExternalOutput").ap()

    inv_sqrt_hd = float(1.0 / np.sqrt(HD))
    rs_groups = [[0, 1, 2, 3], [4, 5, 6, 7]]

    with tile.TileContext(nc) as tc:
        with (
            tc.tile_pool(name="wpool", bufs=1) as wpool,
            tc.tile_pool(name="qkv", bufs=1) as qkv,
            tc.tile_pool(name="nrm", bufs=3) as nrm,
            tc.tile_pool(name="ppool", bufs=4) as ppool,
            tc.tile_pool(name="fpool", bufs=3) as fpool,
            tc.tile_pool(name="psS", bufs=2, space="PSUM") as psS,
            tc.tile_pool(name="psO", bufs=2, space="PSUM") as psO,
            tc.tile_pool(name="shared", bufs=2, space="PSUM") as shared,
            tc.tile_pool(name="dram", bufs=1, space="DRAM") as dram,
        ):
            # ---- weights / constants (DMA-ordered by first use) ----
            wq_sb = wpool.tile([P, KO, D], f16, tag="wq")
            wk_sb = wpool.tile([P, KO, D], f16, tag="wk")
            wv_sb = wpool.tile([P, KO, D], f16, tag="wv")
            wo_sb = wpool.tile([P, D // P, H], f16, tag="wo")
            bq_sb = wpool.tile([P, D // P], f32, tag="bq")
            bk_sb = wpool.tile([P, D // P], f32, tag="bk")
            boeff_sb = wpool.tile([1, H], f32r, tag="boeff")
            mask_sb = wpool.tile([P, P], f16, tag="mask")
            ones_sb = wpool.tile([P, P], f16, tag="ones")
            onesr_sb = wpool.tile([1, P], f32r, tag="onesr")
            bias_sb = wpool.tile([P, H], f32r, tag="bias_bcast")

            xk = wpool.tile([P, KO, T], f16, tag="xk")
            xq = wpool.tile([P, KO, T], f16, tag="xq")
            xv = wpool.tile([P, KO, T], f16, tag="xv")

            def dma_x(x_sb, x_ext, n, kos=(0, KO)):
                # n-th 512-token column chunk of a [H, T] activation
                nc.sync.dma_start(
                    x_sb[:, kos[0] : kos[1], n * 512 : (n + 1) * 512],
                    x_ext.rearrange("(ko p) t -> p ko t", p=P)[
                        :, kos[0] : kos[1], n * 512 : (n + 1) * 512
                    ],
                )

            nc.sync.dma_start(wk_sb[:], wk_ext.rearrange("(ko p) d -> p ko d", p=P))
            dma_x(xk, xk_ext, 0, (0, 4))
            dma_x(xk, xk_ext, 0, (4, KO))
            nc.sync.dma_start(wq_sb[:], wq_ext.rearrange("(ko p) d -> p ko d", p=P))
            dma_x(xq, xq_ext, 0, (0, 4))
            nc.sync.dma_start(bk_sb[:], bk_ext[:])
            dma_x(xq, xq_ext, 0, (4, KO))
            nc.sync.dma_start(bq_sb[:], bq_ext[:])
            nc.sync.dma_start(wv_sb[:], wv_ext.rearrange("(ko p) d -> p ko d", p=P))
            dma_x(xv, xv_ext, 0, (0, 4))
            nc.sync.dma_start(ones_sb[:], ones_ext[:])
            nc.sync.dma_start(mask_sb[:], mask_ext[:])
            dma_x(xv, xv_ext, 0, (4, KO))
            nc.sync.dma_start(onesr_sb[:], onesr_ext[:])
            nc.sync.dma_start(boeff_sb[:], boeff_ext[:])
            nc.sync.dma_start(wo_sb[:], wo_ext.rearrange("(ko p) d -> p ko d", p=P))
            for nn in (1,):
                dma_x(xk, xk_ext, nn)
                dma_x(xq, xq_ext, nn)
                dma_x(xv, xv_ext, nn)

            # ---- persistent per-core tensors ----
            QT = qkv.tile([P, D // P, T], f16, tag="QT")  # [d_par, d_chunk, t]
            KT = qkv.tile([P, D // P, T], f16, tag="KT")
            V = qkv.tile([P, NT, HPC, HD + 1], f16, tag="V")  # [t_par, tk, h, d+1]
            OT = qkv.tile([P, D // P, T], f16, tag="OT")  # normalized attn out
            partial = dram.tile([T, H], f16)  # my heads' w_o contribution
            rs_out = dram.tile([TS, H], f16)  # reduce-scattered sums

            # ---------------- filler units ----------------
            # Each unit is (deadline_key, pe_cost_ns, fn). Deadline keys are
            # (chunk, grp, step) tuples; force(key) pops every unit whose key
            # is <= key (the queue is kept in key order), guaranteeing
            # program-order correctness of projection writes vs their
            # attention readers. meter(budget) pops units to fill the PE
            # slack left by the scalar engine's exp, carrying credit across
            # steps so supply is spread instead of front-loaded.
            fillers = []
            meter_state = {"credit": 0.0}
            NODL = (99, 99, 99)  # no deadline

            def force(key):
                # scan (not just front): prepended no-deadline units may sit
                # ahead of deadline-bearing projection units
                i = 0
                while i < len(fillers):
                    if fillers[i][0] <= key:
                        fillers.pop(i)[2]()
                    else:
                        i += 1

            def meter(budget_ns):
                meter_state["credit"] = min(meter_state["credit"] + budget_ns, 4000.0)
                while fillers and meter_state["credit"] > 0:
                    _, cost, fn = fillers.pop(0)
                    meter_state["credit"] -= cost
                    fn()

            qk_drain_rr = [0]

            def qk_units(x_sb, w_sb, b_sb, OUT, ch, n, key):
                st = {}

                def u1():
                    ps = shared.tile([P, 512], f32, tag="sh", name=f"qk{ch}_{n}")
                    st["ps"] = ps
                    for ko in range(4):
                        nc.tensor.matmul(
                            ps[:],
                            w_sb[:, ko, ch * P : (ch + 1) * P],
                            x_sb[:, ko, n * 512 : (n + 1) * 512],
                            start=(ko == 0),
                            stop=False,
                        )

                def u2():
                    ps = st["ps"]
                    for ko in range(4, KO):
                        nc.tensor.matmul(
                            ps[:],
                            w_sb[:, ko, ch * P : (ch + 1) * P],
                            x_sb[:, ko, n * 512 : (n + 1) * 512],
                            start=False,
                            stop=(ko == KO - 1),
                        )
                    qk_drain_rr[0] ^= 1
                    if qk_drain_rr[0]:
                        nc.vector.tensor_scalar_add(
                            OUT[:, ch, n * 512 : (n + 1) * 512],
                            ps[:],
                            b_sb[:, ch : ch + 1],
                        )
                    else:
                        nc.scalar.activation(
                            OUT[:, ch, n * 512 : (n + 1) * 512],
                            ps[:],
                            AF.Identity,
                            bias=b_sb[:, ch : ch + 1],
                        )

                return [(key, 853.0, u1), (key, 853.0, u2)]

            def v_units(m):
                st = {}
                key = (m // 4, 0, m)  # needed by PV step m of chunk m//4

                def u1():
                    ps = shared.tile([P, 512], f32, tag="sh", name=f"v{m}")[:, :D]
                    st["ps"] = ps
                    for ko in range(4):
                        nc.tensor.matmul(
                            ps[:],
                            xv[:, ko, m * P : (m + 1) * P],
                            wv_sb[:, ko, :],
                            start=(ko == 0),
                            stop=False,
                        )

                def u2():
                    ps = st["ps"]
                    for ko in range(4, KO):
                        nc.tensor.matmul(
                            ps[:],
                            xv[:, ko, m * P : (m + 1) * P],
                            wv_sb[:, ko, :],
                            start=False,
                            stop=(ko == KO - 1),
                        )
                    nc.vector.tensor_copy(
                        V[:, m, :, 0:HD],
                        ps[:].rearrange("p (h d) -> p h d", d=HD),
                    )

                return [(key, 427.0, u1), (key, 427.0, u2)]

            def proj_units_for_chunk(n, part="all"):
                # part "a": group-0 prerequisites (K/Q ch0); "b": the rest,
                # which chunk n itself can absorb as fillers (K/Q ch1 are
                # needed only by group 1, V m-chunks only by PV step m).
                us = []
                if part in ("all", "a"):
                    us += qk_units(xk, wk_sb, bk_sb, KT, 0, n, (n, 0, 0))
                    us += qk_units(xq, wq_sb, bq_sb, QT, 0, n, (n, 0, 0))
                if part in ("all", "b"):
                    us += qk_units(xk, wk_sb, bk_sb, KT, 1, n, (n, 1, 0))
                    us += qk_units(xq, wq_sb, bq_sb, QT, 1, n, (n, 1, 0))
                    for m in range(4 * n, 4 * n + 4):
                        us += v_units(m)
                return us

            def bias_units():
                # bias_bcast[p, :] = boeff for all p (K=1 ones matmul, once)
                def mk(e):
                    def u():
                        ps = shared.tile([P, 512], f32, tag="sh", name=f"biasb{e}")
                        nc.tensor.matmul(
                            ps[:],
                            onesr_sb[:],
                            boeff_sb[:, e * 512 : (e + 1) * 512],
                            start=True,
                            stop=True,
                        )
                        nc.vector.tensor_copy(
                            bias_sb[:, e * 512 : (e + 1) * 512], ps[:]
                        )

                    return u

                return [((0, 1, 0), 427.0, mk(0)), ((0, 1, 0), 427.0, mk(1))]

            def op_units(t, last_of_rs=None):
                # output projection for token chunk t; after e=1, DMA the
                # partial rows out, and optionally launch an RS chunk. The
                # PSUM->SBUF drains (fused with the bias add) alternate
                # between DVE and Pool so neither engine paces the PE stream
                # through the shared-pool rotation.
                st = {}

                n = t // 4
                # op(0)/op(1) must clear before the next chunk's group 1 so
                # their RS fires early; op(2) may spread into late chunk 3,
                # which otherwise starves for PE filler
                key = (n + 1, 1, 0) if n + 2 < NQ else NODL

                last_chunk = t // 4 == NQ - 1

                def mk(e):
                    def u():
                        if e == 0:
                            st["pt"] = fpool.tile(
                                [P, H], f16, tag="pt", name=f"pt{t}"
                            )
                        ps = shared.tile([P, 512], f32, tag="sh", name=f"op{t}_{e}")
                        for ko in range(D // P):
                            nc.tensor.matmul(
                                ps[:],
                                OT[:, ko, t * P : (t + 1) * P],
                                wo_sb[:, ko, e * 512 : (e + 1) * 512],
                                start=(ko == 0),
                                stop=(ko == D // P - 1),
                            )
                        nc.vector.tensor_tensor(
                            st["pt"][:, e * 512 : (e + 1) * 512],
                            ps[:],
                            bias_sb[:, e * 512 : (e + 1) * 512],
                            ALU.add,
                        )
                        if e == 1:
                            nc.sync.dma_start(
                                partial[t * P : (t + 1) * P, :], st["pt"][:]
                            )
                            if last_of_rs is not None:
                                emit_rs(last_of_rs)

                    return u

                return [(key, 426.0, mk(0)), (key, 426.0, mk(1))]

            def emit_rs(n):
                if reps != 1:
                    return
                nc.gpsimd.collective_compute(
                    "ReduceScatter",
                    mybir.AluOpType.add,
                    replica_groups=rs_groups,
                    ins=[partial[512 * n : 512 * (n + 1), :].opt()],
                    outs=[rs_out[128 * n : 128 * (n + 1), :].opt()],
                )

            # ---------------- attention ----------------
            def attn_chunk(n):
                nsl = slice(n * 512, (n + 1) * 512)
                for grp in range(2):
                    ch = grp
                    hA, hB = 2 * grp, 2 * grp + 1
                    psoA = psO.tile([P, 512], f32, tag="psO", name=f"psoA_{n}{grp}")
                    psoB = psO.tile([P, 512], f32, tag="psO", name=f"psoB_{n}{grp}")
                    rsumA = nrm.tile([1, 512], f32r, tag="rsumA", name=f"rsA{n}{grp}")
                    rsumB = nrm.tile([1, 512], f32r, tag="rsumB", name=f"rsB{n}{grp}")
                    imax = 4 * n + 4
                    pend = []  # PV states pending, emitted with lag 2

                    def emit_pv(state):
                        i, c0, PT = state
                        nc.tensor.matmul(
                            psoA[0 : HD + 1, c0:512],
                            V[:, i, hA, :],
                            PT[:, c0:512],
                            start=(i == 0),
                            stop=(i == imax - 1),
                        )
                        nc.tensor.matmul(
                            psoB[0 : HD + 1, c0:512],
                            V[:, i, hB, :],
                            PT[:, 512 + c0 : 1024],
                            start=(i == 0),
                            stop=(i == imax - 1),
                        )

                    for i in range(imax):
                        diag = i >= 4 * n
                        c0 = 128 * (i - 4 * n) if diag else 0
                        force((n, grp, i))
                        pss = psS.tile([P, 1024], f32, tag="psS", name=f"pss{n}{grp}")
                        for po, qt0 in ((0, 0), (64, 512)):
                            nc.tensor.matmul(
                                pss[:, qt0 + c0 : qt0 + 512],
                                KT[po : po + 64, ch, i * P : (i + 1) * P],
                                QT[po : po + 64, ch, n * 512 + c0 : (n + 1) * 512],
                                start=True,
                                stop=True,
                            )
                        act_ns = (1024 - c0) * 0.833 + 355.0
                        if i >= imax - 4:
                            act_ns += 300.0
                        pe_ns = 2 * (512 - c0) * 0.4167
                        if len(pend) >= 2:
                            pv = pend.pop(0)
                            emit_pv(pv)
                            pe_ns += 2 * (512 - pv[1]) * 0.4167
                        meter(act_ns - pe_ns)
                        PT = ppool.tile([P, 1024], f16, tag="PT")
                        nc.scalar.activation(
                            PT[:, c0:1024],
                            pss[:, c0:1024],
                            AF.Exp,
                            scale=inv_sqrt_hd,
                        )
                        if diag:
                            for qt0 in (0, 512):
                                nc.gpsimd.tensor_tensor(
                                    PT[:, qt0 + c0 : qt0 + c0 + 128],
                                    PT[:, qt0 + c0 : qt0 + c0 + 128],
                                    mask_sb[:],
                                    ALU.mult,
                                )
                        pend.append((i, c0, PT))
                    for pv in pend:
                        emit_pv(pv)
                        meter(900.0 - 2 * (512 - pv[1]) * 0.4167)
                    pend = []
                    # fused normalize-and-drain: reciprocals straight off the
                    # pso sums row (DVE + Act in parallel), selector-matmul
                    # broadcast, then the PSUM->SBUF drain IS the 1/sum
                    # multiply (DVE for head A, Pool for head B)
                    with nc.allow_low_precision(
                        reason="softmax denominators tolerate f32r rounding"
                    ):
                        nc.vector.reciprocal(rsumA[:], psoA[HD : HD + 1, :])
                        nc.vector.reciprocal(rsumB[:], psoB[HD : HD + 1, :])

                    nc.vector.tensor_copy(OT[0:64, ch, nsl], psoA[0:HD, :])
                    nc.vector.tensor_copy(OT[64:128, ch, nsl], psoB[0:HD, :])
                    bcA = shared.tile([P, 512], f32, tag="sh", name=f"bcA{n}{grp}")
                    nc.tensor.matmul(
                        bcA[0:64, :], onesr_sb[0:1, 0:64], rsumA[:], start=True, stop=True
                    )
                    bcB = shared.tile([P, 512], f32, tag="sh", name=f"bcB{n}{grp}")
                    nc.tensor.matmul(
                        bcB[0:64, :], onesr_sb[0:1, 0:64], rsumB[:], start=True, stop=True
                    )
                    nc.vector.tensor_tensor(
                        OT[0:64, ch, nsl], OT[0:64, ch, nsl], bcA[0:64, :], ALU.mult
                    )
                    nc.vector.tensor_tensor(
                        OT[64:128, ch, nsl], OT[64:128, ch, nsl], bcB[0:64, :], ALU.mult
                    )
                    meter(600.0)

            def pipeline():
                # pre-phase: minimum projections for chunk 0 group 0
                for _, _, u in qk_units(xk, wk_sb, bk_sb, KT, 0, 0, (0, 0, 0)):
                    u()
                for _, _, u in qk_units(xq, wq_sb, bq_sb, QT, 0, 0, (0, 0, 0)):
                    u()
                for m in range(4):
                    for _, _, u in v_units(m):
                        u()
                nc.vector.tensor_copy(
                    V[:, :, :, HD],
                    ones_sb[:, 0 : NT * HPC].rearrange("p (a b) -> p a b", b=HPC),
                )
                fillers.extend(qk_units(xk, wk_sb, bk_sb, KT, 1, 0, (0, 1, 0)))
                fillers.extend(qk_units(xq, wq_sb, bq_sb, QT, 1, 0, (0, 1, 0)))
                fillers.extend(bias_units())

                for n in range(NQ):
                    if n + 1 < NQ:
                        if n + 2 <= NQ - 1:
                            # x columns for chunk n+2's projections (queued
                            # next iteration) must be in flight before then
                            dma_x(xk, xk_ext, n + 2)
                            dma_x(xq, xq_ext, n + 2)
                            dma_x(xv, xv_ext, n + 2)
                        # the last chunk is filler-starved, so its non-
                        # group-0 projections ride in its own queue
                        part = "all" if n + 1 < NQ - 1 else "a"
                        fillers.extend(proj_units_for_chunk(n + 1, part))
                        if n + 1 == NQ - 1:
                            fillers.extend(proj_units_for_chunk(n + 1, "b"))
                    # attn_chunk queues each group's normalize units itself
                    attn_chunk(n)
                    # output projection + RS ride the queue behind normalize
                    post = []
                    for t in range(4 * n, 4 * n + 4):
                        post += op_units(t, last_of_rs=n if t == 4 * n + 3 else None)
                    if n + 1 < NQ:
                        fillers.extend(post)
                    else:
                        force(NODL)
                        for _, _, u in post:
                            u()
                while fillers:
                    fillers.pop(0)[2]()
                if reps == 1:
                    # single copy depending on ALL four RS chunks: the
                    # scheduler places it topologically last, so SP's
                    # in-order DMA queue never blocks mid-pipeline on a
                    # collective result
                    nc.sync.dma_start(out_ext[:], rs_out[:])

            if reps == 1:
                pipeline()
            else:
                with tc.For_i(0, reps, 1):
                    pipeline()
                nc.sync.dma_start(out_ext[:], partial[0:TS, :])
    nc.finalize()
    return nc


def _host_inputs(q, k, v, w_q, b_q, w_k, b_k, w_v, b_v, w_o, b_o):
    """Shard + lay out the full inputs for the 8 cores."""
    f = np.float32
    h = np.float16
    xT = {}  # (tensor, b) -> [H, T] transposed activations, fp16
    for name, x in (("q", q), ("k", k), ("v", v)):
        for b in range(B):
            xT[(name, b)] = np.ascontiguousarray(np.asarray(x[b], dtype=f).T.astype(h))

    wqT = np.ascontiguousarray(np.asarray(w_q, dtype=f).T.astype(h))
    wkT = np.ascontiguousarray(np.asarray(w_k, dtype=f).T.astype(h))
    wvT = np.ascontiguousarray(np.asarray(w_v, dtype=f).T.astype(h))
    woT = np.ascontiguousarray(np.asarray(w_o, dtype=f).T.astype(h))
    woT32 = np.asarray(w_o, dtype=f).T

    # diagonal-block causal mask: valid iff col >= row
    mask128 = (np.arange(P)[None, :] >= np.arange(P)[:, None]).astype(h)

    b_o32 = np.asarray(b_o, dtype=f)
    b_v32 = np.asarray(b_v, dtype=f)

    in_maps = []
    for c in range(NCORES):
        b, g = divmod(c, GROUPS)
        ds = slice(g * D, (g + 1) * D)
        boeff = b_o32 / GROUPS + b_v32[ds] @ woT32[ds, :]
        in_maps.append(
            {
                "xqT": xT[("q", b)],
                "xkT": xT[("k", b)],
                "xvT": xT[("v", b)],
                "wqT": np.ascontiguousarray(wqT[:, ds]),
                "wkT": np.ascontiguousarray(wkT[:, ds]),
                "wvT": np.ascontiguousarray(wvT[:, ds]),
                "woT": np.ascontiguousarray(woT[ds, :]),
                "bq": np.ascontiguousarray(
                    np.asarray(b_q, dtype=f)[ds].reshape(D // P, P).T
                ),
                "bk": np.ascontiguousarray(
                    np.asarray(b_k, dtype=f)[ds].reshape(D // P, P).T
                ),
                "boeff": boeff.reshape(1, H).astype(f),
                "mask128": mask128,
                "ones": np.ones((P, P), h),
                "onesr": np.ones((1, P), f),
            }
        )
    return in_maps


def kernel(q, k, v, mask, w_q, b_q, w_k, b_k, w_v, b_v, w_o, b_o):
    """Full multi-head attention. mask is always the causal tril mask, which
    the device program hardcodes; the tensor itself is not transferred."""
    from concourse.bass_utils import run_bass_kernel_spmd

    if "nc" not in _nc_cache:
        _nc_cache["nc"] = build_nc()
    nc = _nc_cache["nc"]

    in_maps = _host_inputs(q, k, v, w_q, b_q, w_k, b_k, w_v, b_v, w_o, b_o)
    res = run_bass_kernel_spmd(nc, in_maps, core_ids=list(range(NCORES)))

    out = np.empty((B, T, H), np.float32)
    for c in range(NCORES):
        b, g = divmod(c, GROUPS)
        o = res.results[c]["out"]
        for j in range(4):
            lo = 512 * j + 128 * g
            out[b, lo : lo + 128, :] = o[128 * j : 128 * (j + 1), :]
    return out


# revision 71
# speedup vs baseline: 1.0186x; 1.0186x over previous
"""Multi-head causal attention (B=2, T=2048, H=1024, NH=16) on 8 TRN2 cores.

Sharding: core c owns batch c//4 and heads 4*(c%4)..4*(c%4)+4 (tensor
parallel on heads, data parallel on batch). Each core projects Q/K/V for its
head slice (column parallel), runs causal attention for its 4 heads, applies
its w_o row slice to all tokens, and 4 token-chunked ReduceScatters sum the
partials across each 4-core head group.

Schedule: attention runs tq-chunk-major (4 chunks of 512 query tokens). As
soon as chunk n is attended + normalized + output-projected, its 512-token
ReduceScatter launches and overlaps the compute of chunk n+1; only the last
chunk's RS (15us launch + 0.25MB) plus the final output copy is exposed.
Within each chunk the two heads of a group run in lockstep (one 1024-wide exp
per tk step), PV lags scores by TWO steps (so the only cross-engine gate is
the 2-deep score-PSUM rotation), and Q/K/V/out-projection matmul "fillers"
ride a deadline-keyed, credit-metered queue that injects them between steps
wherever the scalar engine's exp would otherwise stall the tensor engine.

All matmul operands are fp16 (full PE rate at any tile size, so the
below-diagonal halves of boundary score blocks are skipped exactly); softmax
denominators come from an all-ones column appended to V, reciprocated
straight out of PSUM, broadcast by K=1 ones matmuls, and multiplied into the
attention output in place. b_v is folded into an effective output bias
host-side (b_o/4 + b_v_g @ w_oT_g, exact), which a single ones-matmul
broadcasts so the out-projection drain fuses the bias add. PSUM->SBUF drains
alternate between DVE and Act so neither queue paces the PE stream.
"""

import numpy as np

B, T, H, NH, HD = 2, 2048, 1024, 16, 64
NCORES = 8
GROUPS = 4  # head-groups == cores per batch
D = H // GROUPS  # 256 output dims per core
HPC = NH // GROUPS  # 4 heads per core
TS = T // GROUPS  # 512-token output slice per core
P = 128
KO = H // P  # 8 contraction chunks
NQ = T // 512  # 4 tq chunks of 512
NT = T // P  # 16 tk chunks of 128

_nc_cache = {}


def build_nc(reps: int = 1, body: str = "all"):
    """Build the per-core Bass program (identical across cores)."""
    import concourse.mybir as mybir
    import concourse.tile as tile
    from concourse import bacc

    f32 = mybir.dt.float32
    f32r = mybir.dt.float32r
    f16 = mybir.dt.float16
    AF = mybir.ActivationFunctionType
    ALU = mybir.AluOpType

    nc = bacc.Bacc("TRN2", target_bir_lowering=False, debug=False, num_devices=NCORES)

    def inp(name, shape, dt=f16):
        return nc.dram_tensor(name, shape, dt, kind="ExternalInput").ap()

    xq_ext = inp("xqT", [H, T])
    xk_ext = inp("xkT", [H, T])
    xv_ext = inp("xvT", [H, T])
    wq_ext = inp("wqT", [H, D])
    wk_ext = inp("wkT", [H, D])
    wv_ext = inp("wvT", [H, D])
    wo_ext = inp("woT", [D, H])
    bq_ext = inp("bq", [P, D // P], f32)
    bk_ext = inp("bk", [P, D // P], f32)
    boeff_ext = inp("boeff", [1, H], f32r)  # b_o/4 + b_v_g @ w_oT_g
    mask_ext = inp("mask128", [P, P])  # upper-tri (f >= p) diagonal-block mask
    ones_ext = inp("ones", [P, P])
    onesr_ext = inp("onesr", [1, P], f32r)
    out_ext = nc.dram_tensor("out", [TS, H], f16, kind="ExternalOutput").ap()

    inv_sqrt_hd = float(1.0 / np.sqrt(HD))
    rs_groups = [[0, 1, 2, 3], [4, 5, 6, 7]]

    with tile.TileContext(nc) as tc:
        with (
            tc.tile_pool(name="wpool", bufs=1) as wpool,
            tc.tile_pool(name="qkv", bufs=1) as qkv,
            tc.tile_pool(name="nrm", bufs=3) as nrm,
            tc.tile_pool(name="ppool", bufs=4) as ppool,
            tc.tile_pool(name="fpool", bufs=3) as fpool,
            tc.tile_pool(name="psS", bufs=2, space="PSUM") as psS,
            tc.tile_pool(name="psO", bufs=2, space="PSUM") as psO,
            tc.tile_pool(name="shared", bufs=2, space="PSUM") as shared,
            tc.tile_pool(name="dram", bufs=1, space="DRAM") as dram,
        ):
            # ---- weights / constants (DMA-ordered by first use) ----
            wq_sb = wpool.tile([P, KO, D], f16, tag="wq")
            wk_sb = wpool.tile([P, KO, D], f16, tag="wk")
            wv_sb = wpool.tile([P, KO, D], f16, tag="wv")
            wo_sb = wpool.tile([P, D // P, H], f16, tag="wo")
            bq_sb = wpool.tile([P, D // P], f32, tag="bq")
            bk_sb = wpool.tile([P, D // P], f32, tag="bk")
            boeff_sb = wpool.tile([1, H], f32r, tag="boeff")
            mask_sb = wpool.tile([P, P], f16, tag="mask")
            ones_sb = wpool.tile([P, P], f16, tag="ones")
            onesr_sb = wpool.tile([1, P], f32r, tag="onesr")
            bias_sb = wpool.tile([P, H], f32r, tag="bias_bcast")

            xk = wpool.tile([P, KO, T], f16, tag="xk")
            xq = wpool.tile([P, KO, T], f16, tag="xq")
            xv = wpool.tile([P, KO, T], f16, tag="xv")

            def dma_x(x_sb, x_ext, n, kos=(0, KO)):
                # n-th 512-token column chunk of a [H, T] activation
                nc.sync.dma_start(
                    x_sb[:, kos[0] : kos[1], n * 512 : (n + 1) * 512],
                    x_ext.rearrange("(ko p) t -> p ko t", p=P)[
                        :, kos[0] : kos[1], n * 512 : (n + 1) * 512
                    ],
                )

            nc.sync.dma_start(wk_sb[:], wk_ext.rearrange("(ko p) d -> p ko d", p=P))
            dma_x(xk, xk_ext, 0, (0, 4))
            dma_x(xk, xk_ext, 0, (4, KO))
            nc.sync.dma_start(wq_sb[:], wq_ext.rearrange("(ko p) d -> p ko d", p=P))
            dma_x(xq, xq_ext, 0, (0, 4))
            nc.sync.dma_start(bk_sb[:], bk_ext[:])
            dma_x(xq, xq_ext, 0, (4, KO))
            nc.sync.dma_start(bq_sb[:], bq_ext[:])
            nc.sync.dma_start(wv_sb[:], wv_ext.rearrange("(ko p) d -> p ko d", p=P))
            dma_x(xv, xv_ext, 0, (0, 4))
            nc.sync.dma_start(ones_sb[:], ones_ext[:])
            nc.sync.dma_start(mask_sb[:], mask_ext[:])
            dma_x(xv, xv_ext, 0, (4, KO))
            nc.sync.dma_start(onesr_sb[:], onesr_ext[:])
            nc.sync.dma_start(boeff_sb[:], boeff_ext[:])
            nc.sync.dma_start(wo_sb[:], wo_ext.rearrange("(ko p) d -> p ko d", p=P))
            for nn in (1,):
                dma_x(xk, xk_ext, nn)
                dma_x(xq, xq_ext, nn)
                dma_x(xv, xv_ext, nn)

            # ---- persistent per-core tensors ----
            QT = qkv.tile([P, D // P, T], f16, tag="QT")  # [d_par, d_chunk, t]
            KT = qkv.tile([P, D // P, T], f16, tag="KT")
            V = qkv.tile([P, NT, HPC, HD + 1], f16, tag="V")  # [t_par, tk, h, d+1]
            OT = qkv.tile([P, D // P, T], f16, tag="OT")  # normalized attn out
            partial = dram.tile([T, H], f16)  # my heads' w_o contribution
            rs_out = dram.tile([TS, H], f16)  # reduce-scattered sums

            # ---------------- filler units ----------------
            # Each unit is (deadline_key, pe_cost_ns, fn). Deadline keys are
            # (chunk, grp, step) tuples; force(key) pops every unit whose key
            # is <= key (the queue is kept in key order), guaranteeing
            # program-order correctness of projection writes vs their
            # attention readers. meter(budget) pops units to fill the PE
            # slack left by the scalar engine's exp, carrying credit across
            # steps so supply is spread instead of front-loaded.
            fillers = []
            meter_state = {"credit": 0.0}
            NODL = (99, 99, 99)  # no deadline

            def force(key):
                # scan (not just front): prepended no-deadline units may sit
                # ahead of deadline-bearing projection units
                i = 0
                while i < len(fillers):
                    if fillers[i][0] <= key:
                        fillers.pop(i)[2]()
                    else:
                        i += 1

            def meter(budget_ns):
                meter_state["credit"] = min(meter_state["credit"] + budget_ns, 4000.0)
                while fillers and meter_state["credit"] > 0:
                    _, cost, fn = fillers.pop(0)
                    meter_state["credit"] -= cost
                    fn()

            qk_drain_rr = [0]

            def qk_units(x_sb, w_sb, b_sb, OUT, ch, n, key):
                st = {}

                def u1():
                    ps = shared.tile([P, 512], f32, tag="sh", name=f"qk{ch}_{n}")
                    st["ps"] = ps
                    for ko in range(4):
                        nc.tensor.matmul(
                            ps[:],
                            w_sb[:, ko, ch * P : (ch + 1) * P],
                            x_sb[:, ko, n * 512 : (n + 1) * 512],
                            start=(ko == 0),
                            stop=False,
                        )

                def u2():
                    ps = st["ps"]
                    for ko in range(4, KO):
                        nc.tensor.matmul(
                            ps[:],
                            w_sb[:, ko, ch * P : (ch + 1) * P],
                            x_sb[:, ko, n * 512 : (n + 1) * 512],
                            start=False,
                            stop=(ko == KO - 1),
                        )
                    qk_drain_rr[0] ^= 1
                    if qk_drain_rr[0]:
                        nc.vector.tensor_scalar_add(
                            OUT[:, ch, n * 512 : (n + 1) * 512],
                            ps[:],
                            b_sb[:, ch : ch + 1],
                        )
                    else:
                        nc.scalar.activation(
                            OUT[:, ch, n * 512 : (n + 1) * 512],
                            ps[:],
                            AF.Identity,
                            bias=b_sb[:, ch : ch + 1],
                        )

                return [(key, 853.0, u1), (key, 853.0, u2)]

            def v_units(m):
                st = {}
                key = (m // 4, 0, m)  # needed by PV step m of chunk m//4

                def u1():
                    ps = shared.tile([P, 512], f32, tag="sh", name=f"v{m}")[:, :D]
                    st["ps"] = ps
                    for ko in range(4):
                        nc.tensor.matmul(
                            ps[:],
                            xv[:, ko, m * P : (m + 1) * P],
                            wv_sb[:, ko, :],
                            start=(ko == 0),
                            stop=False,
                        )

                def u2():
                    ps = st["ps"]
                    for ko in range(4, KO):
                        nc.tensor.matmul(
                            ps[:],
                            xv[:, ko, m * P : (m + 1) * P],
                            wv_sb[:, ko, :],
                            start=False,
                            stop=(ko == KO - 1),
                        )
                    nc.vector.tensor_copy(
                        V[:, m, :, 0:HD],
                        ps[:].rearrange("p (h d) -> p h d", d=HD),
                    )

                return [(key, 427.0, u1), (key, 427.0, u2)]

            def proj_units_for_chunk(n, part="all"):
                # part "a": group-0 prerequisites (K/Q ch0); "b": the rest,
                # which chunk n itself can absorb as fillers (K/Q ch1 are
                # needed only by group 1, V m-chunks only by PV step m).
                us = []
                # K chunk n's columns are first read at chunk n's DIAGONAL
                # steps (i = 4n), so its deadline is 4n steps looser than Q's
                if part in ("all", "a"):
                    us += qk_units(xq, wq_sb, bq_sb, QT, 0, n, (n, 0, 0))
                    us += qk_units(xk, wk_sb, bk_sb, KT, 0, n, (n, 0, 4 * n))
                if part in ("all", "b"):
                    us += qk_units(xq, wq_sb, bq_sb, QT, 1, n, (n, 1, 0))
                    us += qk_units(xk, wk_sb, bk_sb, KT, 1, n, (n, 1, 4 * n))
                    for m in range(4 * n, 4 * n + 4):
                        us += v_units(m)
                return us

            def bias_units():
                # bias_bcast[p, :] = boeff for all p (K=1 ones matmul, once)
                def mk(e):
                    def u():
                        ps = shared.tile([P, 512], f32, tag="sh", name=f"biasb{e}")
                        nc.tensor.matmul(
                            ps[:],
                            onesr_sb[:],
                            boeff_sb[:, e * 512 : (e + 1) * 512],
                            start=True,
                            stop=True,
                        )
                        nc.vector.tensor_copy(
                            bias_sb[:, e * 512 : (e + 1) * 512], ps[:]
                        )

                    return u

                return [((0, 1, 0), 427.0, mk(0)), ((0, 1, 0), 427.0, mk(1))]

            def op_units(t, last_of_rs=None):
                # output projection for token chunk t; after e=1, DMA the
                # partial rows out, and optionally launch an RS chunk. The
                # PSUM->SBUF drains (fused with the bias add) alternate
                # between DVE and Pool so neither engine paces the PE stream
                # through the shared-pool rotation.
                st = {}

                n = t // 4
                # op(0)/op(1) must clear before the next chunk's group 1 so
                # their RS fires early; op(2) may spread into late chunk 3,
                # which otherwise starves for PE filler
                key = (n + 1, 1, 0) if n + 2 < NQ else NODL

                last_chunk = t // 4 == NQ - 1

                def mk(e):
                    def u():
                        if e == 0:
                            st["pt"] = fpool.tile(
                                [P, H], f16, tag="pt", name=f"pt{t}"
                            )
                        ps = shared.tile([P, 512], f32, tag="sh", name=f"op{t}_{e}")
                        for ko in range(D // P):
                            nc.tensor.matmul(
                                ps[:],
                                OT[:, ko, t * P : (t + 1) * P],
                                wo_sb[:, ko, e * 512 : (e + 1) * 512],
                                start=(ko == 0),
                                stop=(ko == D // P - 1),
                            )
                        nc.vector.tensor_tensor(
                            st["pt"][:, e * 512 : (e + 1) * 512],
                            ps[:],
                            bias_sb[:, e * 512 : (e + 1) * 512],
                            ALU.add,
                        )
                        if e == 1:
                            nc.sync.dma_start(
                                partial[t * P : (t + 1) * P, :], st["pt"][:]
                            )
                            if last_of_rs is not None:
                                emit_rs(last_of_rs)

                    return u

                return [(key, 426.0, mk(0)), (key, 426.0, mk(1))]

            def emit_rs(n):
                if reps != 1:
                    return
                nc.gpsimd.collective_compute(
                    "ReduceScatter",
                    mybir.AluOpType.add,
                    replica_groups=rs_groups,
                    ins=[partial[512 * n : 512 * (n + 1), :].opt()],
                    outs=[rs_out[128 * n : 128 * (n + 1), :].opt()],
                )

            # ---------------- attention ----------------
            def attn_chunk(n):
                nsl = slice(n * 512, (n + 1) * 512)
                for grp in range(2):
                    ch = grp
                    hA, hB = 2 * grp, 2 * grp + 1
                    psoA = psO.tile([P, 512], f32, tag="psO", name=f"psoA_{n}{grp}")
                    psoB = psO.tile([P, 512], f32, tag="psO", name=f"psoB_{n}{grp}")
                    rsumA = nrm.tile([1, 512], f32r, tag="rsumA", name=f"rsA{n}{grp}")
                    rsumB = nrm.tile([1, 512], f32r, tag="rsumB", name=f"rsB{n}{grp}")
                    imax = 4 * n + 4
                    pend = []  # PV states pending, emitted with lag 2

                    def emit_pv(state):
                        i, c0, PT = state
                        nc.tensor.matmul(
                            psoA[0 : HD + 1, c0:512],
                            V[:, i, hA, :],
                            PT[:, c0:512],
                            start=(i == 0),
                            stop=(i == imax - 1),
                        )
                        nc.tensor.matmul(
                            psoB[0 : HD + 1, c0:512],
                            V[:, i, hB, :],
                            PT[:, 512 + c0 : 1024],
                            start=(i == 0),
                            stop=(i == imax - 1),
                        )

                    for i in range(imax):
                        diag = i >= 4 * n
                        c0 = 128 * (i - 4 * n) if diag else 0
                        force((n, grp, i))
                        pss = psS.tile([P, 1024], f32, tag="psS", name=f"pss{n}{grp}")
                        for po, qt0 in ((0, 0), (64, 512)):
                            nc.tensor.matmul(
                                pss[:, qt0 + c0 : qt0 + 512],
                                KT[po : po + 64, ch, i * P : (i + 1) * P],
                                QT[po : po + 64, ch, n * 512 + c0 : (n + 1) * 512],
                                start=True,
                                stop=True,
                            )
                        act_ns = (1024 - c0) * 0.833 + 355.0
                        if i >= imax - 4:
                            act_ns += 300.0
                        pe_ns = 2 * (512 - c0) * 0.4167
                        if len(pend) >= 2:
                            pv = pend.pop(0)
                            emit_pv(pv)
                            pe_ns += 2 * (512 - pv[1]) * 0.4167
                        meter(act_ns - pe_ns)
                        PT = ppool.tile([P, 1024], f16, tag="PT")
                        nc.scalar.activation(
                            PT[:, c0:1024],
                            pss[:, c0:1024],
                            AF.Exp,
                            scale=inv_sqrt_hd,
                        )
                        if diag:
                            for qt0 in (0, 512):
                                nc.gpsimd.tensor_tensor(
                                    PT[:, qt0 + c0 : qt0 + c0 + 128],
                                    PT[:, qt0 + c0 : qt0 + c0 + 128],
                                    mask_sb[:],
                                    ALU.mult,
                                )
                        pend.append((i, c0, PT))
                    for pv in pend:
                        emit_pv(pv)
                        meter(900.0 - 2 * (512 - pv[1]) * 0.4167)
                    pend = []
                    # fused normalize-and-drain: reciprocals straight off the
                    # pso sums row (DVE + Act in parallel), selector-matmul
                    # broadcast, then the PSUM->SBUF drain IS the 1/sum
                    # multiply (DVE for head A, Pool for head B)
                    with nc.allow_low_precision(
                        reason="softmax denominators tolerate f32r rounding"
                    ):
                        nc.vector.reciprocal(rsumA[:], psoA[HD : HD + 1, :])

                    nc.vector.tensor_copy(OT[0:64, ch, nsl], psoA[0:HD, :])
                    with nc.allow_low_precision(
                        reason="softmax denominators tolerate f32r rounding"
                    ):
                        nc.vector.reciprocal(rsumB[:], psoB[HD : HD + 1, :])
                    nc.vector.tensor_copy(OT[64:128, ch, nsl], psoB[0:HD, :])
                    bcA = shared.tile([P, 512], f32, tag="sh", name=f"bcA{n}{grp}")
                    nc.tensor.matmul(
                        bcA[0:64, :], onesr_sb[0:1, 0:64], rsumA[:], start=True, stop=True
                    )
                    bcB = shared.tile([P, 512], f32, tag="sh", name=f"bcB{n}{grp}")
                    nc.tensor.matmul(
                        bcB[0:64, :], onesr_sb[0:1, 0:64], rsumB[:], start=True, stop=True
                    )
                    nc.vector.tensor_tensor(
                        OT[0:64, ch, nsl], OT[0:64, ch, nsl], bcA[0:64, :], ALU.mult
                    )
                    nc.vector.tensor_tensor(
                        OT[64:128, ch, nsl], OT[64:128, ch, nsl], bcB[0:64, :], ALU.mult
                    )
                    meter(600.0)

            def pipeline():
                # pre-phase: minimum projections for chunk 0 group 0
                for _, _, u in qk_units(xk, wk_sb, bk_sb, KT, 0, 0, (0, 0, 0)):
                    u()
                for _, _, u in qk_units(xq, wq_sb, bq_sb, QT, 0, 0, (0, 0, 0)):
                    u()
                for m in range(4):
                    for _, _, u in v_units(m):
                        u()
                nc.vector.tensor_copy(
                    V[:, :, :, HD],
                    ones_sb[:, 0 : NT * HPC].rearrange("p (a b) -> p a b", b=HPC),
                )
                fillers.extend(qk_units(xk, wk_sb, bk_sb, KT, 1, 0, (0, 1, 0)))
                fillers.extend(qk_units(xq, wq_sb, bq_sb, QT, 1, 0, (0, 1, 0)))
                fillers.extend(bias_units())

                for n in range(NQ):
                    if n + 1 < NQ:
                        if n + 2 <= NQ - 1:
                            # x columns for chunk n+2's projections (queued
                            # next iteration) must be in flight before then
                            dma_x(xk, xk_ext, n + 2)
                            dma_x(xq, xq_ext, n + 2)
                            dma_x(xv, xv_ext, n + 2)
                        # the last chunk is filler-starved, so its non-
                        # group-0 projections ride in its own queue
                        part = "all" if n + 1 < NQ - 1 else "a"
                        fillers.extend(proj_units_for_chunk(n + 1, part))
                        if n + 1 == NQ - 1:
                            fillers.extend(proj_units_for_chunk(n + 1, "b"))
                    # attn_chunk queues each group's normalize units itself
                    attn_chunk(n)
                    # output projection + RS ride the queue behind normalize
                    post = []
                    for t in range(4 * n, 4 * n + 4):
                        post += op_units(t, last_of_rs=n if t == 4 * n + 3 else None)
                    if n + 1 < NQ:
                        fillers.extend(post)
                    else:
                        force(NODL)
                        for _, _, u in post:
                            u()
                while fillers:
                    fillers.pop(0)[2]()
                if reps == 1:
                    # single copy depending on ALL four RS chunks: the
                    # scheduler places it topologically last, so SP's
                    # in-order DMA queue never blocks mid-pipeline on a
                    # collective result
                    nc.sync.dma_start(out_ext[0:384, :], rs_out[0:384, :])
                    nc.sync.dma_start(out_ext[384:512, :], rs_out[384:512, :])

            if reps == 1:
                pipeline()
            else:
                with tc.For_i(0, reps, 1):
                    pipeline()
                nc.sync.dma_start(out_ext[:], partial[0:TS, :])
    nc.finalize()
    return nc


def _host_inputs(q, k, v, w_q, b_q, w_k, b_k, w_v, b_v, w_o, b_o):
    """Shard + lay out the full inputs for the 8 cores."""
    f = np.float32
    h = np.float16
    xT = {}  # (tensor, b) -> [H, T] transposed activations, fp16
    for name, x in (("q", q), ("k", k), ("v", v)):
        for b in range(B):
            xT[(name, b)] = np.ascontiguousarray(np.asarray(x[b], dtype=f).T.astype(h))

    wqT = np.ascontiguousarray(np.asarray(w_q, dtype=f).T.astype(h))
    wkT = np.ascontiguousarray(np.asarray(w_k, dtype=f).T.astype(h))
    wvT = np.ascontiguousarray(np.asarray(w_v, dtype=f).T.astype(h))
    woT = np.ascontiguousarray(np.asarray(w_o, dtype=f).T.astype(h))
    woT32 = np.asarray(w_o, dtype=f).T

    # diagonal-block causal mask: valid iff col >= row
    mask128 = (np.arange(P)[None, :] >= np.arange(P)[:, None]).astype(h)

    b_o32 = np.asarray(b_o, dtype=f)
    b_v32 = np.asarray(b_v, dtype=f)

    in_maps = []
    for c in range(NCORES):
        b, g = divmod(c, GROUPS)
        ds = slice(g * D, (g + 1) * D)
        boeff = b_o32 / GROUPS + b_v32[ds] @ woT32[ds, :]
        in_maps.append(
            {
                "xqT": xT[("q", b)],
                "xkT": xT[("k", b)],
                "xvT": xT[("v", b)],
                "wqT": np.ascontiguousarray(wqT[:, ds]),
                "wkT": np.ascontiguousarray(wkT[:, ds]),
                "wvT": np.ascontiguousarray(wvT[:, ds]),
                "woT": np.ascontiguousarray(woT[ds, :]),
                "bq": np.ascontiguousarray(
                    np.asarray(b_q, dtype=f)[ds].reshape(D // P, P).T
                ),
                "bk": np.ascontiguousarray(
                    np.asarray(b_k, dtype=f)[ds].reshape(D // P, P).T
                ),
                "boeff": boeff.reshape(1, H).astype(f),
                "mask128": mask128,
                "ones": np.ones((P, P), h),
                "onesr": np.ones((1, P), f),
            }
        )
    return in_maps


def kernel(q, k, v, mask, w_q, b_q, w_k, b_k, w_v, b_v, w_o, b_o):
    """Full multi-head attention. mask is always the causal tril mask, which
    the device program hardcodes; the tensor itself is not transferred."""
    from concourse.bass_utils import run_bass_kernel_spmd

    if "nc" not in _nc_cache:
        _nc_cache["nc"] = build_nc()
    nc = _nc_cache["nc"]

    in_maps = _host_inputs(q, k, v, w_q, b_q, w_k, b_k, w_v, b_v, w_o, b_o)
    res = run_bass_kernel_spmd(nc, in_maps, core_ids=list(range(NCORES)))

    out = np.empty((B, T, H), np.float32)
    for c in range(NCORES):
        b, g = divmod(c, GROUPS)
        o = res.results[c]["out"]
        for j in range(4):
            lo = 512 * j + 128 * g
            out[b, lo : lo + 128, :] = o[128 * j : 128 * (j + 1), :]
    return out
